# revision 41
# baseline (speedup 1.0000x reference)
"""Causal self-attention with ALiBi on 8 Trainium2 NeuronCores.

Sharding: batch x heads. Cores 0-3 own batch 0, cores 4-7 batch 1; within a
batch group core g owns heads {g, g+4, g+8, g+12}, processed in "slots"
0..3 = heads g+12, g+8, g+4, g. Each core computes its head-slice of the qkv
projection, attention for its 4 heads, and a partial out-projection
(row-split of W_out over its heads' feature dims); the host sums the 4
partial outputs per batch (the TP all-reduce, done on host since outputs are
gathered anyway).

Banded attention: ALiBi mass decays as exp(-slope*dist), so tk blocks with
slope*dist >> 1 are dropped. Per-slot band profiles X = [12, 6, 2, 1] extra
128-row blocks before the diagonal 512-col tile; the slot assignment above
pairs each profile with heads whose slope makes the dropped mass < ~e^-14 of
the softmax denominator (slot 0 drops nothing). This balances work across
cores exactly (identical program; head identity lives in the data).

Numerics: the qkv projection runs in fp8 (e4m3) DoubleRow mode at 2x the
bf16 matmul rate, with a 3-term error-compensated split prepared on the
host: x ~ x_hi + x_lo and W ~ W_hi + W_lo (each the fp8 quantization and its
fp8-quantized residual), accumulating x_hi*W_hi + x_lo*W_hi + x_hi*W_lo in
fp32 PSUM. This is ~4x more accurate than a bf16 projection. Weight slices
are pre-scaled by powers of two (q: 2^8 including the 1/sqrt(hd) attention
scale, k/v: 2^5) so fp8's narrow exponent range is used well; the PSUM->SBUF
copies divide the scale back out (free: activation/tensor_scalar with
immediate scale). Attention stays bf16: the ALiBi bias -slope*(tq-tk) rides
inside the QK^T matmul as 4 extra contraction rows using an exact hi/lo
split of the position index, causal masking of diagonal 128-row blocks adds
a -480 triangle into PSUM via an fp8 DoubleRow identity matmul (exp then
flushes those entries to 0 in fp32), softmax skips the max subtraction
(logits are bounded), exp runs on ScalarE straight out of PSUM. P^T @ V uses
V augmented with a ones column so row 64 of the PV accumulator is the
softmax denominator; normalization is a DVE reciprocal (to bf16) + a K=1
broadcast matmul + DVE elementwise multiply. The out-projection accumulates
its two 128-row contraction chunks directly in PSUM, and P tiles are
consumed by PV three steps after production (pipelined across slot
boundaries) so the qk->exp->pv latency stays off the PE critical path.
"""

import copy
import math

import ml_dtypes
import numpy as np

import concourse.bass as bass
import concourse.mybir as mybir
import concourse.tile as tile

from concourse.bass_utils import run_bass_kernel_spmd

BF16 = mybir.dt.bfloat16
F32 = mybir.dt.float32
FP8 = mybir.dt.float8e4
NPBF16 = ml_dtypes.bfloat16
NPF8 = ml_dtypes.float8_e4m3
DR = mybir.MatmulPerfMode.DoubleRow

B, T, D, H = 2, 2048, 1024, 16
HD = D // H  # 64
NCORES = 8
HPC = 4  # heads per core (batch-split: 4 cores per batch element)
P = 128
TQ = 512  # query tile width
NKB = T // P  # 16 tk blocks
NCT = T // TQ  # 4 query tiles
KC = D // P  # 8 contraction chunks for the projections
KCP = KC // 2  # 4 DoubleRow chunk-pairs
KA = HD + 4  # 68: contraction rows for QK^T (features + 4 ALiBi-bias rows)
FPC = HPC * HD  # 256 feature dims per core
EC = FPC // P  # 2 e-tile chunks of the out-projection contraction
XS = [12, 6, 2, 1]  # per-slot extra tk blocks before the diagonal tile

SCL_Q = 2.0**8  # host pre-scale on W_q (incl. 1/sqrt(hd)); copies undo it
SCL_K = 2.0**5
SCL_V = 2.0**5

# ---------------------------------------------------------------------------
# Workaround for this container's walrus build: engine-queue instructions
# accept only ONE sync-wait command. Tile attaches several; split the extras
# onto NoOps inserted just before the instruction on the same engine.
# ---------------------------------------------------------------------------


def _split_multiwait_instructions(nc):
    for f in nc.m.functions:
        for bb in f.blocks:
            insts = bb.instructions
            i = 0
            while i < len(insts):
                inst = insts[i]
                si = inst.sync_info
                waits = list(si.on_wait) if si is not None else []
                if len(waits) > 1:
                    si_keep = copy.deepcopy(si)
                    si_keep.on_wait = waits[-1:]
                    inst.sync_info = si_keep
                    for w in waits[:-1]:
                        nop = mybir.InstNoOp(
                            name=nc.get_next_instruction_name(), ins=[], outs=[]
                        )
                        nop.engine = inst.engine
                        nsi = copy.deepcopy(si)
                        nsi.on_wait = [w]
                        nsi.on_update = []
                        nop.sync_info = nsi
                        nc.register_instruction(nop, overwrite=True)
                        insts.insert(i, nop)
                        i += 1
                i += 1


_patch_done = False


def _apply_tile_patch():
    global _patch_done
    if _patch_done:
        return
    orig = tile.TileContext.schedule_and_allocate

    def patched(self, *args, **kwargs):
        ret = orig(self, *args, **kwargs)
        _split_multiwait_instructions(self.nc)
        return ret

    tile.TileContext.schedule_and_allocate = patched
    _patch_done = True


# ---------------------------------------------------------------------------
# Bass program (identical on all cores; per-core data differs)
# ---------------------------------------------------------------------------


def _build_nc():
    _apply_tile_patch()
    nc = bass.Bass()

    xhid = nc.dram_tensor("xhi", [D, T], FP8, kind="ExternalInput")
    xlod = nc.dram_tensor("xlo", [D, T], FP8, kind="ExternalInput")
    whid = nc.dram_tensor("whi", [D, 3 * FPC], FP8, kind="ExternalInput")
    wlod = nc.dram_tensor("wlo", [D, 3 * FPC], FP8, kind="ExternalInput")
    augd = nc.dram_tensor("aug", [4, 2 * HPC, T], BF16, kind="ExternalInput")
    woutd = nc.dram_tensor("woutT", [FPC, D], BF16, kind="ExternalInput")
    eye8d = nc.dram_tensor("eye8", [P, 2, P], FP8, kind="ExternalInput")
    tri8d = nc.dram_tensor("tri8", [P, 2, P], FP8, kind="ExternalInput")
    onesd = nc.dram_tensor("ones64", [1, HD], BF16, kind="ExternalInput")
    ytd = nc.dram_tensor("yT", [D, T], BF16, kind="ExternalOutput")

    EXP = mybir.ActivationFunctionType.Exp

    with tile.TileContext(nc) as tc:
        with (
            tc.tile_pool(name="consts", bufs=1) as consts,
            tc.tile_pool(name="xtp", bufs=1) as xtp,
            tc.tile_pool(name="qkp", bufs=1) as qkp,
            tc.tile_pool(name="vp", bufs=1) as vp,
            tc.tile_pool(name="aop", bufs=1) as aop,
            tc.tile_pool(name="pp", bufs=14) as pp,
            tc.tile_pool(name="yp", bufs=3) as yp,
            tc.tile_pool(name="rp", bufs=4) as rp,
            tc.tile_pool(name="bcp", bufs=3) as bcp,
            tc.tile_pool(name="psA", bufs=2, space="PSUM") as psA,
            tc.tile_pool(name="psY", bufs=2, space="PSUM") as psY,
        ):
            whi_sb = consts.tile([P, KC, 3 * FPC], FP8)
            wlo_sb = consts.tile([P, KC, 3 * FPC], FP8)
            wo_sb = consts.tile([P, EC, D], BF16)
            eye8_sb = consts.tile([P, 2, P], FP8)
            tri8_sb = consts.tile([P, 2, P], FP8)
            ones_sb = consts.tile([1, HD], BF16)

            ytr = ytd.rearrange("(e p) t -> p e t", p=P)

            # ---- loads: w/x chunk-pair groups first (the boot projection
            # consumes chunk-pairs in order hi*hi, lo*hi, hi*lo); spread
            # issue cost across the SP / Act / DVE queues ----
            xhi = xtp.tile([P, KC, T], FP8, name="xhi_sb")
            xlo = xtp.tile([P, KC, T], FP8, name="xlo_sb")
            # q/k tensors fused into one [68, 8, T] tile: index 0-3 = q slot,
            # 4-7 = k slot. Lets the 8 bias-aug row loads be a single DMA.
            qk_all = qkp.tile([KA, 2 * HPC, T], BF16, name="qk_all_sb")
            qh = [qk_all[:, h] for h in range(HPC)]
            kh = [qk_all[:, HPC + h] for h in range(HPC)]
            xhir = xhid.rearrange("(kc p) t -> p kc t", p=P)
            xlor = xlod.rearrange("(kc p) t -> p kc t", p=P)
            whir = whid.rearrange("(kc p) e -> p kc e", p=P)
            wlor = wlod.rearrange("(kc p) e -> p kc e", p=P)
# The DMA channel is serialized, so issue in consumption order:
            # the weights, then x token-slab 0 (the boot projects q/k for
            # the thin slots 2-3 one token-slab at a time), then the consts
            # the first attention steps touch (aug bias rows, mask), then
            # the remaining x slabs, then the out-projection weight.
# The DMA channel is serialized, so issue in consumption order:
            # per chunk-pair all 4 tensors (sync carries the hi pair, scalar
            # the lo pair so the queues interleave), then the bias-aug rows
            # (needed by the first QK right after boot) and the small consts.
            for kcp in range(KCP):
                sl = slice(2 * kcp, 2 * kcp + 2)
                nc.sync.dma_start(whi_sb[:, sl, :], whir[:, sl, :])
                nc.sync.dma_start(xhi[:, sl, :], xhir[:, sl, :])
                nc.scalar.dma_start(xlo[:, sl, :], xlor[:, sl, :])
                nc.scalar.dma_start(wlo_sb[:, sl, :], wlor[:, sl, :])
            nc.scalar.dma_start(qk_all[HD:KA, :, :], augd[:])
            nc.sync.dma_start(eye8_sb[:], eye8d[:])
            nc.sync.dma_start(tri8_sb[:], tri8d[:])
            nc.sync.dma_start(ones_sb[:], onesd[:])
            nc.sync.dma_start(wo_sb[:], woutd.rearrange("(ck p) e -> p ck e", p=P))
            V = vp.tile([P, NKB, HPC, HD + 1], BF16, name="V_sb")
            nc.vector.memset(V[:, :, :, HD : HD + 1], 1.0)

            def dr_terms(ecols):
                """The 12 (lhsT, rhs) chunk-pair operands of one compensated
                fp8 projection group over weight columns ``ecols``.
                Chunk-pair-major: the DMA stream is serialized, so each
                chunk-pair's 3 terms (2.6us of boot PE work across the 8
                groups) should unlock as soon as that pair's 4 tensors (4us
                of DMA) land."""
                ops = []
                for kcp in range(KCP):
                    sl = slice(2 * kcp, 2 * kcp + 2)
                    ops.append((whi_sb[:, sl, ecols], xhi[:, sl]))
                    ops.append((whi_sb[:, sl, ecols], xlo[:, sl]))
                    ops.append((wlo_sb[:, sl, ecols], xhi[:, sl]))
                return ops


            # ---- q/k projection groups (feature-major) ----
            def proj_group(et, t4, pool=None, tag=None):
                """One [128-feature x 512-token] projection group.
                et: 0=q s01, 1=q s23, 2=k s01, 3=k s23."""
                tgt = qh if et < EC else kh
                scl = 1.0 / SCL_Q if et < EC else 1.0 / SCL_K
                hbase = (et % EC) * 2
                ps = (pool or psY).tile(
                    [P, TQ], F32, tag=(tag or "psY"), name="ps_qk"
                )
                ops = dr_terms(slice(et * P, (et + 1) * P))
                tsl = slice(t4 * TQ, (t4 + 1) * TQ)
                for i, (wop, xop) in enumerate(ops):
                    nc.tensor.matmul(
                        ps[:],
                        wop,
                        xop[:, :, tsl],
                        start=(i == 0),
                        stop=(i == len(ops) - 1),
                        perf_mode=DR,
                    )
                nc.vector.tensor_scalar_mul(tgt[hbase][0:HD, tsl], ps[0:HD, :], scl)
                nc.scalar.mul(tgt[hbase + 1][0:HD, tsl], ps[HD:P, :], scl)

            # upfront: q and k for slots 0-1 only (slots 2-3 fill slot-1's
            # loop). All 8 accumulation groups stay open (8 PSUM banks via a
            # scoped boot pool) and the matmuls are emitted CHUNK-PAIR-major,
            # so the PE consumes each w/x chunk-pair across all groups the
            # moment its DMA lands instead of stalling once per group.
            with tc.tile_pool(name="psBoot", bufs=4, space="PSUM") as psBoot:
                boot = [(et, t4) for et in (0, 2) for t4 in range(NCT)]
                pss = []
                for i in range(len(boot)):
                    pool, tag = [
                        (psY, "psY"), (psA, "psA"),
                        (psBoot, "psBoot"), (psBoot, "psBoot"),
                    ][i % 4]
                    pss.append(pool.tile([P, TQ], F32, tag=tag, name="ps_qk"))
                nterm = 3 * KCP
                for ti in range(nterm):
                    # last pass in copy-drain order so groups 0/4 (slot-0's
                    # first q/k tiles) close their accumulation first
                    idxs = (0, 4, 1, 5, 2, 6, 3, 7) if ti == nterm - 1 else range(8)
                    for i in idxs:
                        et, t4 = boot[i]
                        ops = dr_terms(slice(et * P, (et + 1) * P))
                        wop, xop = ops[ti]
                        nc.tensor.matmul(
                            pss[i][:],
                            wop,
                            xop[:, :, t4 * TQ : (t4 + 1) * TQ],
                            start=(ti == 0),
                            stop=(ti == nterm - 1),
                            perf_mode=DR,
                        )
                # psY-resident groups (0, 4) drain first so slot-0's
                # v_groups can claim those banks without waiting on the
                # whole copy queue
                for i in (0, 4, 1, 5, 2, 6, 3, 7):
                    et, t4 = boot[i]
                    tgt = qh if et < EC else kh
                    scl = 1.0 / SCL_Q if et < EC else 1.0 / SCL_K
                    hbase = (et % EC) * 2
                    tsl = slice(t4 * TQ, (t4 + 1) * TQ)
                    nc.vector.tensor_scalar_mul(
                        tgt[hbase][0:HD, tsl], pss[i][0:HD, :], scl
                    )
                    nc.scalar.mul(tgt[hbase + 1][0:HD, tsl], pss[i][HD:P, :], scl)

            def v_group(t16):
                """One token-major v-projection block (PE filler in slot 0)."""
                psv = psY.tile([P, TQ], F32, tag="psY", name="ps_v")
                vcols = slice(2 * FPC, 3 * FPC)
                i = 0
                for kcp in range(KCP):
                    sl = slice(2 * kcp, 2 * kcp + 2)
                    tsl = slice(t16 * P, (t16 + 1) * P)
                    for wop, xop in (
                        (whi_sb[:, sl, vcols], xhi[:, sl, tsl]),
                        (whi_sb[:, sl, vcols], xlo[:, sl, tsl]),
                        (wlo_sb[:, sl, vcols], xhi[:, sl, tsl]),
                    ):
                        nc.tensor.matmul(
                            psv[:, 0:FPC],
                            xop,
                            wop,
                            start=(i == 0),
                            stop=(i == 3 * KCP - 1),
                            perf_mode=DR,
                        )
                        i += 1
                nc.vector.tensor_scalar_mul(
                    V[:, t16, :, 0:HD],
                    psv[:, 0:FPC].rearrange("p (h f) -> p h f", h=HPC),
                    1.0 / SCL_V,
                )

            ao01 = aop.tile([P, T], BF16, tag="ao01", name="ao01_sb")
            ao23 = aop.tile([P, T], BF16, tag="ao23", name="ao23_sb")

            def outproj(c, pool2=None, n2=0):
                """Both 128-row contraction chunks accumulate in PSUM.
                With only the 2 psY banks the matmul stream is paced by the
                PSUM->SBUF copies (~675ns/e-tile vs 426ns of matmul), so up
                to ``n2`` odd e-tiles borrow banks from the pv pool — only
                as many as have provably been released by their normalize at
                this point in the program order (an unreleased one would
                deadlock the in-order PE queue). Copies alternate DVE/Act;
                the last slice streams out in quarter-DMAs to cut the
                end-of-kernel drain."""
                EH = D // P // 2  # 4 e-tiles per output half
                used2 = 0
                for half in range(2):
                    ysb = yp.tile([P, EH, TQ], BF16, tag="y", name="y_sb")
                    for i in range(EH):
                        e = half * EH + i
                        if pool2 is not None and e % 2 == 1 and used2 < n2:
                            used2 += 1
                            yps = pool2.tile([P, TQ], F32, tag="pv", name="ps_y2")
                        else:
                            yps = psY.tile([P, TQ], F32, tag="psY", name="ps_y")
                        nc.tensor.matmul(
                            yps[:],
                            wo_sb[:, 0, e * P : (e + 1) * P],
                            ao01[:, c * TQ : (c + 1) * TQ],
                            start=True,
                            stop=False,
                        )
                        nc.tensor.matmul(
                            yps[:],
                            wo_sb[:, 1, e * P : (e + 1) * P],
                            ao23[:, c * TQ : (c + 1) * TQ],
                            start=False,
                            stop=True,
                        )
                        if e % 2 == 0:
                            nc.vector.tensor_copy(ysb[:, i, :], yps[:])
                        else:
                            nc.scalar.copy(ysb[:, i, :], yps[:])
                        if c == NCT - 1 and i % 2 == 1:
                            nc.sync.dma_start(
                                ytr[
                                    :,
                                    half * EH + i - 1 : half * EH + i + 1,
                                    c * TQ : (c + 1) * TQ,
                                ],
                                ysb[:, i - 1 : i + 1, :],
                            )
                    if c < NCT - 1:
                        nc.sync.dma_start(
                            ytr[
                                :, half * EH : (half + 1) * EH, c * TQ : (c + 1) * TQ
                            ],
                            ysb[:],
                        )

            # ---- attention: one flat software pipeline across all four
            # head-slots. P tiles are consumed by PV two steps after
            # production (hides qk->exp->pv latency), and the pipeline is
            # CARRIED across slot boundaries: slot h's first QK tiles issue
            # while slot h-1's last PV / normalize work drains, so the PE
            # never idles at a slot transition. pv PSUM banks rotate safely:
            # a slot's pv[c] bank is last read at its normalize, 11+ steps
            # before the next slot's pv[c] first write.
            psB = tc.alloc_tile_pool(name="psB", bufs=4, space="PSUM")
            pvs = {}

            def normalize(hh, c):
                """Per-(slot, c-tile) softmax normalization. The reciprocal
                goes to bf16 so the K=1 broadcast matmul streams at full
                rate (213ns vs 853ns for fp32); ~0.2% scale error, well
                inside budget."""
                ao = ao01 if hh < 2 else ao23
                hrow = (hh % 2) * HD
                rc = rp.tile([1, TQ], BF16, tag="rc", name="rc_sb")
                with nc.allow_low_precision(
                    reason="bf16 softmax-denominator reciprocal: "
                    "0.2% scale error, tolerance is 2e-2"
                ):
                    nc.vector.reciprocal(rc[:], pvs[hh][c][HD : HD + 1, :])
                bc_ps = psA.tile([HD, TQ], F32, tag="psA", name="ps_bc")
                nc.tensor.matmul(
                    bc_ps[:], ones_sb[:], rc[:], start=True, stop=True
                )
                bc_sb = bcp.tile([HD, TQ], F32, tag="bcs", name="bc_sb")
                nc.vector.tensor_copy(bc_sb[:], bc_ps[:])
                nc.vector.tensor_mul(
                    ao[hrow : hrow + HD, c * TQ : (c + 1) * TQ],
                    pvs[hh][c][0:HD, :],
                    bc_sb[:],
                )

            LAG = 3  # steps between P-tile production (exp) and PV use
            pipe = [[] for _ in range(LAG)]
            horder = (0, 1, 2, 3)  # fat band first: it hosts the v-proj
            # fillers under its exp-bound steps; the thin slot 3 runs last
            # and hosts the out-projection.
            for hpos, h in enumerate(horder):
                X = XS[h]
                pvs[h] = [
                    psB.tile([HD + 1, TQ], F32, tag="pv", name=f"pv{h}_{c}")
                    for c in range(NCT)
                ]
                steps = NKB + (LAG if hpos == HPC - 1 else 0)
                for a in range(steps):
                    cur = []
                    if a < NKB:
                        c0 = a // 4
                        cmax = min(NCT - 1, (a + X) // 4)
                        for c in range(c0, cmax + 1):
                            diag = c == c0
                            off = P * (a % 4) if diag else 0
                            qk = psA.tile([P, TQ], F32, tag="psA", name="ps_s")
                            nc.tensor.matmul(
                                qk[:, off:TQ],
                                kh[h][:, a * P : (a + 1) * P],
                                qh[h][:, c * TQ + off : (c + 1) * TQ],
                                start=True,
                                stop=not diag,
                            )
                            if diag:
                                nc.tensor.matmul(
                                    qk[:, off : off + P],
                                    eye8_sb[:],
                                    tri8_sb[:],
                                    start=False,
                                    stop=True,
                                    perf_mode=DR,
                                    skip_group_check=True,
                                )
                            pt = pp.tile([P, TQ], BF16, tag="psb", name="p_sb")
                            nc.scalar.activation(pt[:, off:TQ], qk[:, off:TQ], EXP)
                            cur.append((h, a, c, off, pt))
                    for (hh, aa, c, off, pt) in pipe[0]:
                        nc.tensor.matmul(
                            pvs[hh][c][:, off:TQ],
                            V[:, aa, hh, :],
                            pt[:, off:TQ],
                            start=(aa == max(0, 4 * c - XS[hh])),
                            stop=(aa == 4 * c + 3),
                        )
                    pipe = pipe[1:] + [cur]
                    # fillers AFTER the critical QK/PV work of the step
                    if hpos == 0 and a < NKB:
                        v_group(a)
                    if hpos == 1 and a < 2 * NCT:
                        proj_group(1 if a < NCT else 3, a % NCT)
                    # previous slot's last c-tile completes LAG steps into
                    # this slot; its own c<3 tiles complete at a=4c+LAG+2
                    if a == LAG - 1 and hpos > 0:
                        normalize(horder[hpos - 1], NCT - 1)
                    if a >= LAG + 3 and (a - LAG + 1) % 4 == 0:
                        normalize(h, (a - LAG - 1) // 4)
                    # out-projection rides the tail of the last-executed
                    # slot, a few iterations behind each slice's norm chain
                    if hpos == 3 and a in (7, 11, 15):
                        outproj({7: 0, 11: 1, 15: 2}[a], pool2=psB, n2=1)
            outproj(3, pool2=psB, n2=4)
            psB.release()

    return nc


_NC_CACHE = {}


def _get_nc():
    if "nc" not in _NC_CACHE:
        _NC_CACHE["nc"] = _build_nc()
    return _NC_CACHE["nc"]


# ---------------------------------------------------------------------------
# Host side: shard, run, gather
# ---------------------------------------------------------------------------


def _f8_split(a):
    hi = a.astype(NPF8)
    lo = (a - hi.astype(np.float32)).astype(NPF8)
    return hi, lo


def _make_in_maps(x, W_qkv, W_out, n_heads):
    ratio = 2.0 ** (-8.0 / n_heads)
    slopes = np.asarray([ratio ** (i + 1) for i in range(n_heads)], np.float32)
    scale = np.float32(1.0 / math.sqrt(D // n_heads))

    xt = np.ascontiguousarray(x.transpose(0, 2, 1)).astype(np.float32)  # [B, D, T]
    xt_hi = [None] * B
    xt_lo = [None] * B
    for b in range(B):
        xt_hi[b], xt_lo[b] = _f8_split(xt[b])

    t = np.arange(T, dtype=np.float32)
    t_hi = np.floor(t / 16.0).astype(np.float32)  # 0..127, exact in bf16
    t_lo = (t - 16.0 * t_hi).astype(np.float32)  # 0..15, exact in bf16

    tri = np.where(
        np.arange(P)[None, :] >= np.arange(P)[:, None], np.float32(0.0),
        np.float32(-240.0),
    ).astype(NPF8)
    eye8 = np.zeros((P, 2, P), NPF8)
    tri8 = np.zeros((P, 2, P), NPF8)
    eye8[:, 0, :] = np.eye(P, dtype=NPF8)
    eye8[:, 1, :] = np.eye(P, dtype=NPF8)
    tri8[:, 0, :] = tri
    tri8[:, 1, :] = tri
    ones64 = np.ones((1, HD), NPBF16)

    in_maps = []
    for core in range(NCORES):
        b = core // (NCORES // B)  # cores 0-3 -> batch 0, 4-7 -> batch 1
        g = core % (NCORES // B)
        # slot s processes head g + 4*(3-s): full band profiles take the
        # small-slope heads, narrow profiles the steep-slope heads
        hs = [g + 12, g + 8, g + 4, g]
        wq = np.concatenate(
            [W_qkv[h * HD : (h + 1) * HD, :] for h in hs], 0
        ) * (scale * np.float32(SCL_Q))
        wk = np.concatenate(
            [W_qkv[D + h * HD : D + (h + 1) * HD, :] for h in hs], 0
        ) * np.float32(SCL_K)
        wv = np.concatenate(
            [W_qkv[2 * D + h * HD : 2 * D + (h + 1) * HD, :] for h in hs], 0
        ) * np.float32(SCL_V)
        wqkvT = np.ascontiguousarray(
            np.concatenate([wq, wk, wv], 0).T
        ).astype(np.float32)  # [D, 3*FPC]
        w_hi, w_lo = _f8_split(wqkvT)
        woutT = np.ascontiguousarray(
            np.concatenate(
                [W_out[:, h * HD : (h + 1) * HD] for h in hs], 1
            ).T
        ).astype(NPBF16)  # [FPC, D]

        aug = np.zeros((4, 2 * HPC, T), np.float32)
        for i, h in enumerate(hs):
            s_bf = np.float32(NPBF16(slopes[h]))
            s16 = np.float32(16.0) * s_bf
            aug[0, i] = t_hi
            aug[1, i] = t_lo
            aug[2, i] = s16
            aug[3, i] = s_bf
            aug[0, HPC + i] = -s16
            aug[1, HPC + i] = -s_bf
            aug[2, HPC + i] = t_hi
            aug[3, HPC + i] = t_lo

        in_maps.append(
            {
                "xhi": xt_hi[b],
                "xlo": xt_lo[b],
                "whi": w_hi,
                "wlo": w_lo,
                "woutT": woutT,
                "aug": aug.astype(NPBF16),
                "eye8": eye8,
                "tri8": tri8,
                "ones64": ones64,
            }
        )
    return in_maps


def _run(x, W_qkv, W_out, n_heads, **spmd_kwargs):
    x = np.asarray(x, dtype=np.float32)
    W_qkv = np.asarray(W_qkv, dtype=np.float32)
    W_out = np.asarray(W_out, dtype=np.float32)
    n_heads = int(n_heads)
    assert x.shape == (B, T, D) and n_heads == H

    in_maps = _make_in_maps(x, W_qkv, W_out, n_heads)
    res = run_bass_kernel_spmd(
        _get_nc(), in_maps, core_ids=list(range(NCORES)), **spmd_kwargs
    )
    gpb = NCORES // B
    y = np.empty((B, T, D), np.float32)
    for b in range(B):
        acc = np.zeros((D, T), np.float32)
        for g in range(gpb):
            acc += np.asarray(res.results[b * gpb + g]["yT"], np.float32)
        y[b] = acc.T
    return y, res


def kernel(x, W_qkv, W_out, n_heads):
    y, _ = _run(x, W_qkv, W_out, n_heads)
    if not np.isfinite(y).all():
        # rare transient device fault observed on this setup; one retry
        y, _ = _run(x, W_qkv, W_out, n_heads)
    return y


# revision 55
# speedup vs baseline: 1.0560x; 1.0560x over previous
"""Causal self-attention with ALiBi on 8 Trainium2 NeuronCores.

Sharding: batch x heads. Cores 0-3 own batch 0, cores 4-7 batch 1; within a
batch group core g owns heads {g, g+4, g+8, g+12}, processed in "slots"
0..3 = heads g+12, g+8, g+4, g. Each core computes its head-slice of the qkv
projection, attention for its 4 heads, and a partial out-projection
(row-split of W_out over its heads' feature dims); the host sums the 4
partial outputs per batch (the TP all-reduce, done on host since outputs are
gathered anyway).

Banded attention: ALiBi mass decays as exp(-slope*dist), so tk blocks with
slope*dist >> 1 are dropped. Per-slot band profiles X = [12, 6, 2, 1] extra
128-row blocks before the diagonal 512-col tile; the slot assignment above
pairs each profile with heads whose slope makes the dropped mass < ~e^-14 of
the softmax denominator (slot 0 drops nothing). This balances work across
cores exactly (identical program; head identity lives in the data).

Numerics: the qkv projection runs in fp8 (e4m3) DoubleRow mode at 2x the
bf16 matmul rate, with a 3-term error-compensated split prepared on the
host: x ~ x_hi + x_lo and W ~ W_hi + W_lo (each the fp8 quantization and its
fp8-quantized residual), accumulating x_hi*W_hi + x_lo*W_hi + x_hi*W_lo in
fp32 PSUM. This is ~4x more accurate than a bf16 projection. Weight slices
are pre-scaled by powers of two (q: 2^8 including the 1/sqrt(hd) attention
scale, k/v: 2^5) so fp8's narrow exponent range is used well; the PSUM->SBUF
copies divide the scale back out (free: activation/tensor_scalar with
immediate scale). Attention stays bf16: the ALiBi bias -slope*(tq-tk) rides
inside the QK^T matmul as 4 extra contraction rows using an exact hi/lo
split of the position index, causal masking of diagonal 128-row blocks adds
a -480 triangle into PSUM via an fp8 DoubleRow identity matmul (exp then
flushes those entries to 0 in fp32), softmax skips the max subtraction
(logits are bounded), exp runs on ScalarE straight out of PSUM. P^T @ V uses
V augmented with a ones column so row 64 of the PV accumulator is the
softmax denominator; normalization is a DVE reciprocal (to bf16) + a K=1
broadcast matmul + DVE elementwise multiply. The out-projection accumulates
its two 128-row contraction chunks directly in PSUM, and P tiles are
consumed by PV three steps after production (pipelined across slot
boundaries) so the qk->exp->pv latency stays off the PE critical path.
"""

import copy
import math

import ml_dtypes
import numpy as np

import concourse.bass as bass
import concourse.mybir as mybir
import concourse.tile as tile

from concourse.bass_utils import run_bass_kernel_spmd

BF16 = mybir.dt.bfloat16
F32 = mybir.dt.float32
FP8 = mybir.dt.float8e4
NPBF16 = ml_dtypes.bfloat16
NPF8 = ml_dtypes.float8_e4m3
DR = mybir.MatmulPerfMode.DoubleRow

B, T, D, H = 2, 2048, 1024, 16
HD = D // H  # 64
NCORES = 8
HPC = 4  # heads per core (batch-split: 4 cores per batch element)
P = 128
TQ = 512  # query tile width
NKB = T // P  # 16 tk blocks
NCT = T // TQ  # 4 query tiles
KC = D // P  # 8 contraction chunks for the projections
KCP = KC // 2  # 4 DoubleRow chunk-pairs
KA = HD + 4  # 68: contraction rows for QK^T (features + 4 ALiBi-bias rows)
FPC = HPC * HD  # 256 feature dims per core
EC = FPC // P  # 2 e-tile chunks of the out-projection contraction
XS = [12, 6, 2, 1]  # per-slot extra tk blocks before the diagonal tile

SCL_Q = 2.0**8  # host pre-scale on W_q (incl. 1/sqrt(hd)); copies undo it
SCL_K = 2.0**5
SCL_V = 2.0**5

# ---------------------------------------------------------------------------
# Workaround for this container's walrus build: engine-queue instructions
# accept only ONE sync-wait command. Tile attaches several; split the extras
# onto NoOps inserted just before the instruction on the same engine.
# ---------------------------------------------------------------------------


def _split_multiwait_instructions(nc):
    for f in nc.m.functions:
        for bb in f.blocks:
            insts = bb.instructions
            i = 0
            while i < len(insts):
                inst = insts[i]
                si = inst.sync_info
                waits = list(si.on_wait) if si is not None else []
                if len(waits) > 1:
                    si_keep = copy.deepcopy(si)
                    si_keep.on_wait = waits[-1:]
                    inst.sync_info = si_keep
                    for w in waits[:-1]:
                        nop = mybir.InstNoOp(
                            name=nc.get_next_instruction_name(), ins=[], outs=[]
                        )
                        nop.engine = inst.engine
                        nsi = copy.deepcopy(si)
                        nsi.on_wait = [w]
                        nsi.on_update = []
                        nop.sync_info = nsi
                        nc.register_instruction(nop, overwrite=True)
                        insts.insert(i, nop)
                        i += 1
                i += 1


_patch_done = False


def _apply_tile_patch():
    global _patch_done
    if _patch_done:
        return
    orig = tile.TileContext.schedule_and_allocate

    def patched(self, *args, **kwargs):
        ret = orig(self, *args, **kwargs)
        _split_multiwait_instructions(self.nc)
        return ret

    tile.TileContext.schedule_and_allocate = patched
    _patch_done = True


# ---------------------------------------------------------------------------
# Bass program (identical on all cores; per-core data differs)
# ---------------------------------------------------------------------------


def _build_nc():
    _apply_tile_patch()
    nc = bass.Bass()

    xhid = nc.dram_tensor("xhi", [D, T], FP8, kind="ExternalInput")
    xlod = nc.dram_tensor("xlo", [D, T], FP8, kind="ExternalInput")
    whid = nc.dram_tensor("whi", [D, 3 * FPC], FP8, kind="ExternalInput")
    wlod = nc.dram_tensor("wlo", [D, 3 * FPC], FP8, kind="ExternalInput")
    augd = nc.dram_tensor("aug", [4, 2 * HPC, T], BF16, kind="ExternalInput")
    woutd = nc.dram_tensor("woutT", [FPC, D], BF16, kind="ExternalInput")
    eye8d = nc.dram_tensor("eye8", [P, 2, P], FP8, kind="ExternalInput")
    tri8d = nc.dram_tensor("tri8", [P, 2, P], FP8, kind="ExternalInput")
    onesd = nc.dram_tensor("ones64", [1, HD], BF16, kind="ExternalInput")
    ytd = nc.dram_tensor("yT", [D, T], BF16, kind="ExternalOutput")

    EXP = mybir.ActivationFunctionType.Exp

    with tile.TileContext(nc) as tc:
        with (
            tc.tile_pool(name="consts", bufs=1) as consts,
            tc.tile_pool(name="xtp", bufs=1) as xtp,
            tc.tile_pool(name="qkp", bufs=1) as qkp,
            tc.tile_pool(name="vp", bufs=1) as vp,
            tc.tile_pool(name="aop", bufs=1) as aop,
            tc.tile_pool(name="pp", bufs=18) as pp,
            tc.tile_pool(name="yp", bufs=3) as yp,
            tc.tile_pool(name="rp", bufs=4) as rp,
            tc.tile_pool(name="bcp", bufs=3) as bcp,
            tc.tile_pool(name="psA", bufs=2, space="PSUM") as psA,
            tc.tile_pool(name="psY", bufs=2, space="PSUM") as psY,
        ):
            whi_sb = consts.tile([P, KC, 3 * FPC], FP8)
            wlo_sb = consts.tile([P, KC, 3 * FPC], FP8)
            wo_sb = consts.tile([P, EC, D], BF16)
            eye8_sb = consts.tile([P, 2, P], FP8)
            tri8_sb = consts.tile([P, 2, P], FP8)
            ones_sb = consts.tile([1, HD], BF16)

            ytr = ytd.rearrange("(e p) t -> p e t", p=P)

            # ---- loads: w/x chunk-pair groups first (the boot projection
            # consumes chunk-pairs in order hi*hi, lo*hi, hi*lo); spread
            # issue cost across the SP / Act / DVE queues ----
            xhi = xtp.tile([P, KC, T], FP8, name="xhi_sb")
            xlo = xtp.tile([P, KC, T], FP8, name="xlo_sb")
            # q/k tensors fused into one [68, 8, T] tile: index 0-3 = q slot,
            # 4-7 = k slot. Lets the 8 bias-aug row loads be a single DMA.
            qk_all = qkp.tile([KA, 2 * HPC, T], BF16, name="qk_all_sb")
            qh = [qk_all[:, h] for h in range(HPC)]
            kh = [qk_all[:, HPC + h] for h in range(HPC)]
            xhir = xhid.rearrange("(kc p) t -> p kc t", p=P)
            xlor = xlod.rearrange("(kc p) t -> p kc t", p=P)
            whir = whid.rearrange("(kc p) e -> p kc e", p=P)
            wlor = wlod.rearrange("(kc p) e -> p kc e", p=P)
# The DMA channel is serialized, so issue in consumption order:
            # the weights, then x token-slab 0 (the boot projects q/k for
            # the thin slots 2-3 one token-slab at a time), then the consts
            # the first attention steps touch (aug bias rows, mask), then
            # the remaining x slabs, then the out-projection weight.
# The DMA channel is serialized, so issue in consumption order:
            # per chunk-pair all 4 tensors (sync carries the hi pair, scalar
            # the lo pair so the queues interleave), then the bias-aug rows
            # (needed by the first QK right after boot) and the small consts.
            qkc = slice(0, 2 * FPC)  # boot needs only the q/k w-columns;
            vc = slice(2 * FPC, 3 * FPC)  # v-columns stream after
            for kcp in range(KCP):
                sl = slice(2 * kcp, 2 * kcp + 2)
                nc.sync.dma_start(whi_sb[:, sl, qkc], whir[:, sl, qkc])
                nc.sync.dma_start(xhi[:, sl, :], xhir[:, sl, :])
                nc.scalar.dma_start(xlo[:, sl, :], xlor[:, sl, :])
                nc.scalar.dma_start(wlo_sb[:, sl, qkc], wlor[:, sl, qkc])
            nc.scalar.dma_start(qk_all[HD:KA, :, :], augd[:])
            nc.sync.dma_start(whi_sb[:, :, vc], whir[:, :, vc])
            nc.scalar.dma_start(wlo_sb[:, :, vc], wlor[:, :, vc])
            nc.sync.dma_start(eye8_sb[:], eye8d[:])
            nc.sync.dma_start(tri8_sb[:], tri8d[:])
            nc.sync.dma_start(ones_sb[:], onesd[:])
            nc.sync.dma_start(wo_sb[:], woutd.rearrange("(ck p) e -> p ck e", p=P))
            V = vp.tile([P, NKB, HPC, HD + 1], BF16, name="V_sb")
            nc.vector.memset(V[:, :, :, HD : HD + 1], 1.0)

            def dr_terms(ecols):
                """The 12 (lhsT, rhs) chunk-pair operands of one compensated
                fp8 projection group over weight columns ``ecols``.
                Chunk-pair-major: the DMA stream is serialized, so each
                chunk-pair's 3 terms (2.6us of boot PE work across the 8
                groups) should unlock as soon as that pair's 4 tensors (4us
                of DMA) land."""
                ops = []
                for kcp in range(KCP):
                    sl = slice(2 * kcp, 2 * kcp + 2)
                    ops.append((whi_sb[:, sl, ecols], xhi[:, sl]))
                    ops.append((whi_sb[:, sl, ecols], xlo[:, sl]))
                    ops.append((wlo_sb[:, sl, ecols], xhi[:, sl]))
                return ops


            # ---- q/k projection groups (feature-major) ----
            def proj_group(et, t4, pool=None, tag=None, cross_dve=False):
                """One [128-feature x 512-token] projection group.
                et: 0=q s01, 1=q s23, 2=k s01, 3=k s23."""
                tgt = qh if et < EC else kh
                scl = 1.0 / SCL_Q if et < EC else 1.0 / SCL_K
                hbase = (et % EC) * 2
                ps = (pool or psY).tile(
                    [P, TQ], F32, tag=(tag or "psY"), name="ps_qk"
                )
                ops = dr_terms(slice(et * P, (et + 1) * P))
                tsl = slice(t4 * TQ, (t4 + 1) * TQ)
                for i, (wop, xop) in enumerate(ops):
                    nc.tensor.matmul(
                        ps[:],
                        wop,
                        xop[:, :, tsl],
                        start=(i == 0),
                        stop=(i == len(ops) - 1),
                        perf_mode=DR,
                    )
                nc.vector.tensor_scalar_mul(tgt[hbase][0:HD, tsl], ps[0:HD, :], scl)
                if cross_dve:
                    nc.vector.tensor_scalar_mul(
                        tgt[hbase + 1][0:HD, tsl], ps[HD:P, :], scl
                    )
                else:
                    nc.scalar.mul(tgt[hbase + 1][0:HD, tsl], ps[HD:P, :], scl)

            # upfront: q and k for slots 0-1 only (slots 2-3 fill slot-1's
            # loop). All 8 accumulation groups stay open (8 PSUM banks via a
            # scoped boot pool) and the matmuls are emitted CHUNK-PAIR-major,
            # so the PE consumes each w/x chunk-pair across all groups the
            # moment its DMA lands instead of stalling once per group.
            with tc.tile_pool(name="psBoot", bufs=4, space="PSUM") as psBoot:
                boot = [(et, t4) for et in (0, 2) for t4 in range(NCT)]
                pss = []
                for i in range(len(boot)):
                    pool, tag = [
                        (psY, "psY"), (psA, "psA"),
                        (psBoot, "psBoot"), (psBoot, "psBoot"),
                    ][i % 4]
                    pss.append(pool.tile([P, TQ], F32, tag=tag, name="ps_qk"))
                nterm = 3 * KCP
                for ti in range(nterm):
                    # last pass in copy-drain order so groups 0/4 (slot-0's
                    # first q/k tiles) close their accumulation first
                    idxs = (0, 4, 1, 2, 3, 5, 6, 7) if ti == nterm - 1 else range(8)
                    for i in idxs:
                        et, t4 = boot[i]
                        ops = dr_terms(slice(et * P, (et + 1) * P))
                        wop, xop = ops[ti]
                        nc.tensor.matmul(
                            pss[i][:],
                            wop,
                            xop[:, :, t4 * TQ : (t4 + 1) * TQ],
                            start=(ti == 0),
                            stop=(ti == nterm - 1),
                            perf_mode=DR,
                        )
                # psY-resident groups (0, 4) drain first so slot-0's
                # v_groups can claim those banks without waiting on the
                # whole copy queue
                for i in (0, 4, 1, 2, 3, 5, 6, 7):
                    et, t4 = boot[i]
                    tgt = qh if et < EC else kh
                    scl = 1.0 / SCL_Q if et < EC else 1.0 / SCL_K
                    hbase = (et % EC) * 2
                    tsl = slice(t4 * TQ, (t4 + 1) * TQ)
                    nc.vector.tensor_scalar_mul(
                        tgt[hbase][0:HD, tsl], pss[i][0:HD, :], scl
                    )
                    nc.scalar.mul(tgt[hbase + 1][0:HD, tsl], pss[i][HD:P, :], scl)

            def v_group(t16):
                """One token-major v-projection block (PE filler in slot 0)."""
                psv = psY.tile([P, TQ], F32, tag="psY", name="ps_v")
                vcols = slice(2 * FPC, 3 * FPC)
                i = 0
                for kcp in range(KCP):
                    sl = slice(2 * kcp, 2 * kcp + 2)
                    tsl = slice(t16 * P, (t16 + 1) * P)
                    for wop, xop in (
                        (whi_sb[:, sl, vcols], xhi[:, sl, tsl]),
                        (whi_sb[:, sl, vcols], xlo[:, sl, tsl]),
                        (wlo_sb[:, sl, vcols], xhi[:, sl, tsl]),
                    ):
                        nc.tensor.matmul(
                            psv[:, 0:FPC],
                            xop,
                            wop,
                            start=(i == 0),
                            stop=(i == 3 * KCP - 1),
                            perf_mode=DR,
                        )
                        i += 1
                nc.vector.tensor_scalar_mul(
                    V[:, t16, :, 0:HD],
                    psv[:, 0:FPC].rearrange("p (h f) -> p h f", h=HPC),
                    1.0 / SCL_V,
                )

            ao01 = aop.tile([P, T], BF16, tag="ao01", name="ao01_sb")
            ao23 = aop.tile([P, T], BF16, tag="ao23", name="ao23_sb")

            def outproj(c, pool2=None, n2=0):
                """Both 128-row contraction chunks accumulate in PSUM.
                With only the 2 psY banks the matmul stream is paced by the
                PSUM->SBUF copies (~675ns/e-tile vs 426ns of matmul), so up
                to ``n2`` odd e-tiles borrow banks from the pv pool — only
                as many as have provably been released by their normalize at
                this point in the program order (an unreleased one would
                deadlock the in-order PE queue). Copies alternate DVE/Act;
                the last slice streams out in quarter-DMAs to cut the
                end-of-kernel drain."""
                EH = D // P // 2  # 4 e-tiles per output half
                used2 = 0
                for half in range(2):
                    ysb = yp.tile([P, EH, TQ], BF16, tag="y", name="y_sb")
                    for i in range(EH):
                        e = half * EH + i
                        if pool2 is not None and e % 2 == 1 and used2 < n2:
                            used2 += 1
                            yps = pool2.tile([P, TQ], F32, tag="pv", name="ps_y2")
                        else:
                            yps = psY.tile([P, TQ], F32, tag="psY", name="ps_y")
                        nc.tensor.matmul(
                            yps[:],
                            wo_sb[:, 0, e * P : (e + 1) * P],
                            ao01[:, c * TQ : (c + 1) * TQ],
                            start=True,
                            stop=False,
                        )
                        nc.tensor.matmul(
                            yps[:],
                            wo_sb[:, 1, e * P : (e + 1) * P],
                            ao23[:, c * TQ : (c + 1) * TQ],
                            start=False,
                            stop=True,
                        )
                        if e % 2 == 0:
                            nc.vector.tensor_copy(ysb[:, i, :], yps[:])
                        else:
                            nc.scalar.copy(ysb[:, i, :], yps[:])
                        if c == NCT - 1 and i % 2 == 1:
                            nc.sync.dma_start(
                                ytr[
                                    :,
                                    half * EH + i - 1 : half * EH + i + 1,
                                    c * TQ : (c + 1) * TQ,
                                ],
                                ysb[:, i - 1 : i + 1, :],
                            )
                    if c < NCT - 1:
                        nc.sync.dma_start(
                            ytr[
                                :, half * EH : (half + 1) * EH, c * TQ : (c + 1) * TQ
                            ],
                            ysb[:],
                        )

            # ---- attention: one flat software pipeline across all four
            # head-slots. P tiles are consumed by PV two steps after
            # production (hides qk->exp->pv latency), and the pipeline is
            # CARRIED across slot boundaries: slot h's first QK tiles issue
            # while slot h-1's last PV / normalize work drains, so the PE
            # never idles at a slot transition. pv PSUM banks rotate safely:
            # a slot's pv[c] bank is last read at its normalize, 11+ steps
            # before the next slot's pv[c] first write.
            psB = tc.alloc_tile_pool(name="psB", bufs=4, space="PSUM")
            pvs = {}

            def normalize(hh, c, pool=None):
                """Per-(slot, c-tile) softmax normalization. The reciprocal
                goes to bf16 so the K=1 broadcast matmul streams at full
                rate (213ns vs 853ns for fp32); ~0.2% scale error, well
                inside budget."""
                ao = ao01 if hh < 2 else ao23
                hrow = (hh % 2) * HD
                rc = rp.tile([1, TQ], BF16, tag="rc", name="rc_sb")
                with nc.allow_low_precision(
                    reason="bf16 softmax-denominator reciprocal: "
                    "0.2% scale error, tolerance is 2e-2"
                ):
                    nc.vector.reciprocal(rc[:], pvs[hh][c][HD : HD + 1, :])
                bc_ps = (pool or psA).tile(
                    [HD, TQ], F32, tag=("psY" if pool is psY else "psA"),
                    name="ps_bc",
                )
                nc.tensor.matmul(
                    bc_ps[:], ones_sb[:], rc[:], start=True, stop=True
                )
                bc_sb = bcp.tile([HD, TQ], F32, tag="bcs", name="bc_sb")
                nc.vector.tensor_copy(bc_sb[:], bc_ps[:])
                nc.vector.tensor_mul(
                    ao[hrow : hrow + HD, c * TQ : (c + 1) * TQ],
                    pvs[hh][c][0:HD, :],
                    bc_sb[:],
                )

            LAG = 3  # steps between P-tile production (exp) and PV use
            pipe = [[] for _ in range(LAG)]
            horder = (0, 1, 2, 3)  # fat band first: it hosts the v-proj
            # fillers under its exp-bound steps; the thin slot 3 runs last
            # and hosts the out-projection.
            for hpos, h in enumerate(horder):
                X = XS[h]
                pvs[h] = [
                    psB.tile([HD + 1, TQ], F32, tag="pv", name=f"pv{h}_{c}")
                    for c in range(NCT)
                ]
                steps = NKB + (LAG if hpos == HPC - 1 else 0)
                for a in range(steps):
                    # slot 0's first steps: v-projections BEFORE the QKs,
                    # filling the PE while the boot's q/k copies drain
                    if hpos == 0 and a < 3:
                        v_group(a)
                    cur = []
                    if a < NKB:
                        c0 = a // 4
                        cmax = min(NCT - 1, (a + X) // 4)
                        for c in range(c0, cmax + 1):
                            diag = c == c0
                            off = P * (a % 4) if diag else 0
                            qkpool, qktag = (
                                (psY, "psY")
                                if hpos == 2 and c % 2 == 1
                                else (psA, "psA")
                            )
                            qk = qkpool.tile([P, TQ], F32, tag=qktag, name="ps_s")
                            nc.tensor.matmul(
                                qk[:, off:TQ],
                                kh[h][:, a * P : (a + 1) * P],
                                qh[h][:, c * TQ + off : (c + 1) * TQ],
                                start=True,
                                stop=not diag,
                            )
                            if diag:
                                nc.tensor.matmul(
                                    qk[:, off : off + P],
                                    eye8_sb[:],
                                    tri8_sb[:],
                                    start=False,
                                    stop=True,
                                    perf_mode=DR,
                                    skip_group_check=True,
                                )
                            pt = pp.tile([P, TQ], BF16, tag="psb", name="p_sb")
                            nc.scalar.activation(pt[:, off:TQ], qk[:, off:TQ], EXP)
                            cur.append((h, a, c, off, pt))
                    for (hh, aa, c, off, pt) in pipe[0]:
                        nc.tensor.matmul(
                            pvs[hh][c][:, off:TQ],
                            V[:, aa, hh, :],
                            pt[:, off:TQ],
                            start=(aa == max(0, 4 * c - XS[hh])),
                            stop=(aa == 4 * c + 3),
                        )
                    pipe = pipe[1:] + [cur]
                    # fillers AFTER the critical QK/PV work of the step
                    if hpos == 0 and 3 <= a < NKB:
                        v_group(a)
                    if hpos == 1 and a < 2 * NCT:
                        proj_group(1 if a < NCT else 3, a % NCT, cross_dve=True)
                    # previous slot's last c-tile completes LAG steps into
                    # this slot; its own c<3 tiles complete at a=4c+LAG+2
                    # during slot 2 (and slot 3's first steps) psY is
                    # idle: borrow it for the normalize broadcast so the
                    # qk double-buffer keeps both psA banks
                    npool = psY if (hpos != 3 or a < 7) else None
                    if a == LAG - 1 and hpos > 0:
                        normalize(horder[hpos - 1], NCT - 1, pool=npool)
                    if a >= LAG + 3 and (a - LAG + 1) % 4 == 0:
                        normalize(h, (a - LAG - 1) // 4, pool=npool)
                    # out-projection rides the tail of the last-executed
                    # slot, a few iterations behind each slice's norm chain
                    if hpos == 3 and a in (7, 11, 15):
                        outproj({7: 0, 11: 1, 15: 2}[a], pool2=psB, n2=1)
            outproj(3, pool2=psB, n2=4)
            psB.release()

    return nc


_NC_CACHE = {}


def _get_nc():
    if "nc" not in _NC_CACHE:
        _NC_CACHE["nc"] = _build_nc()
    return _NC_CACHE["nc"]


# ---------------------------------------------------------------------------
# Host side: shard, run, gather
# ---------------------------------------------------------------------------


def _f8_split(a):
    hi = a.astype(NPF8)
    lo = (a - hi.astype(np.float32)).astype(NPF8)
    return hi, lo


def _make_in_maps(x, W_qkv, W_out, n_heads):
    ratio = 2.0 ** (-8.0 / n_heads)
    slopes = np.asarray([ratio ** (i + 1) for i in range(n_heads)], np.float32)
    scale = np.float32(1.0 / math.sqrt(D // n_heads))

    xt = np.ascontiguousarray(x.transpose(0, 2, 1)).astype(np.float32)  # [B, D, T]
    xt_hi = [None] * B
    xt_lo = [None] * B
    for b in range(B):
        xt_hi[b], xt_lo[b] = _f8_split(xt[b])

    t = np.arange(T, dtype=np.float32)
    t_hi = np.floor(t / 16.0).astype(np.float32)  # 0..127, exact in bf16
    t_lo = (t - 16.0 * t_hi).astype(np.float32)  # 0..15, exact in bf16

    tri = np.where(
        np.arange(P)[None, :] >= np.arange(P)[:, None], np.float32(0.0),
        np.float32(-240.0),
    ).astype(NPF8)
    eye8 = np.zeros((P, 2, P), NPF8)
    tri8 = np.zeros((P, 2, P), NPF8)
    eye8[:, 0, :] = np.eye(P, dtype=NPF8)
    eye8[:, 1, :] = np.eye(P, dtype=NPF8)
    tri8[:, 0, :] = tri
    tri8[:, 1, :] = tri
    ones64 = np.ones((1, HD), NPBF16)

    in_maps = []
    for core in range(NCORES):
        b = core // (NCORES // B)  # cores 0-3 -> batch 0, 4-7 -> batch 1
        g = core % (NCORES // B)
        # slot s processes head g + 4*(3-s): full band profiles take the
        # small-slope heads, narrow profiles the steep-slope heads
        hs = [g + 12, g + 8, g + 4, g]
        wq = np.concatenate(
            [W_qkv[h * HD : (h + 1) * HD, :] for h in hs], 0
        ) * (scale * np.float32(SCL_Q))
        wk = np.concatenate(
            [W_qkv[D + h * HD : D + (h + 1) * HD, :] for h in hs], 0
        ) * np.float32(SCL_K)
        wv = np.concatenate(
            [W_qkv[2 * D + h * HD : 2 * D + (h + 1) * HD, :] for h in hs], 0
        ) * np.float32(SCL_V)
        wqkvT = np.ascontiguousarray(
            np.concatenate([wq, wk, wv], 0).T
        ).astype(np.float32)  # [D, 3*FPC]
        w_hi, w_lo = _f8_split(wqkvT)
        woutT = np.ascontiguousarray(
            np.concatenate(
                [W_out[:, h * HD : (h + 1) * HD] for h in hs], 1
            ).T
        ).astype(NPBF16)  # [FPC, D]

        aug = np.zeros((4, 2 * HPC, T), np.float32)
        for i, h in enumerate(hs):
            s_bf = np.float32(NPBF16(slopes[h]))
            s16 = np.float32(16.0) * s_bf
            aug[0, i] = t_hi
            aug[1, i] = t_lo
            aug[2, i] = s16
            aug[3, i] = s_bf
            aug[0, HPC + i] = -s16
            aug[1, HPC + i] = -s_bf
            aug[2, HPC + i] = t_hi
            aug[3, HPC + i] = t_lo

        in_maps.append(
            {
                "xhi": xt_hi[b],
                "xlo": xt_lo[b],
                "whi": w_hi,
                "wlo": w_lo,
                "woutT": woutT,
                "aug": aug.astype(NPBF16),
                "eye8": eye8,
                "tri8": tri8,
                "ones64": ones64,
            }
        )
    return in_maps


def _run(x, W_qkv, W_out, n_heads, **spmd_kwargs):
    x = np.asarray(x, dtype=np.float32)
    W_qkv = np.asarray(W_qkv, dtype=np.float32)
    W_out = np.asarray(W_out, dtype=np.float32)
    n_heads = int(n_heads)
    assert x.shape == (B, T, D) and n_heads == H

    in_maps = _make_in_maps(x, W_qkv, W_out, n_heads)
    res = run_bass_kernel_spmd(
        _get_nc(), in_maps, core_ids=list(range(NCORES)), **spmd_kwargs
    )
    gpb = NCORES // B
    y = np.empty((B, T, D), np.float32)
    for b in range(B):
        acc = np.zeros((D, T), np.float32)
        for g in range(gpb):
            acc += np.asarray(res.results[b * gpb + g]["yT"], np.float32)
        y[b] = acc.T
    return y, res


def kernel(x, W_qkv, W_out, n_heads):
    y, _ = _run(x, W_qkv, W_out, n_heads)
    if not np.isfinite(y).all():
        # rare transient device fault observed on this setup; one retry
        y, _ = _run(x, W_qkv, W_out, n_heads)
    return y
